# revision 1
# baseline (speedup 1.0000x reference)
"""DeepseekMoE block (attention + top-2 routed MoE + shared expert) on 8 TRN2
NeuronCores, data-parallel over the batch dimension (B=8 -> one batch per core).

Layout strategy per core (L=1024 tokens, H=1024 hidden):
  - Activations live in "F-layout" [feature-on-partitions, tokens-on-free] so
    every matmul chains without transposes (weights are pre-transposed on host
    to [K_in, M_out]).
  - Per-token scalars (rms scales, softmax 1/Z, gate weights, output gate) are
    produced as [1, L] rows and broadcast across partitions with K=1 rank-1
    matmuls on the TensorEngine.
  - Attention is computed transposed (attT[k, q]) so the key-padding mask and
    exp() fold into one scalar-engine activation (bias = per-partition mask
    column), and ctx comes out of the pT@V matmul directly in F-layout.
  - Precision tiers: float32r (fp32 truncated to fp22, full matmul speed) for
    the QKV/out_proj projections, exact fp32 for the router logits (top-2
    selection is chaotically sensitive), bf16 for attention scores/probs and
    the expert FFNs.
"""

import numpy as np
import ml_dtypes
from contextlib import ExitStack

import concourse.bass as bass
import concourse.mybir as mybir
import concourse.tile as tile
from concourse import bacc
from concourse.bass_utils import run_bass_kernel_spmd
from concourse.masks import make_identity

B, L, H = 8, 1024, 1024
E, I, NH, HD = 8, 256, 4, 256
ISZ = 512
P = 128
NT = L // P      # token blocks
KH = H // P      # hidden slabs
ND = HD // P     # d-blocks per head (=2)
EPS = 1e-6
NEG = -30000.0
INV_SQRT_HD = float(1.0 / np.sqrt(HD))

DT = mybir.dt
F32, BF16, I32 = DT.float32, DT.bfloat16, DT.int32
F32R = DT.float32r
Alu = mybir.AluOpType
Act = mybir.ActivationFunctionType
AX = mybir.AxisListType


def build():
    nc = bacc.Bacc("TRN2", target_bir_lowering=False, debug=False)

    def din(name, shape, dt):
        return nc.dram_tensor(name, shape, dt, kind="ExternalInput").ap()

    xT = din("x_t", [H, L], F32)
    tcc = din("tc_col", [P, 1], F32)
    wqk = din("wqkT", [H, 2 * H], F32R)
    wvm = din("wvT", [H, H], F32R)
    wom = din("woT", [H, H], F32R)
    wgm = din("wgT", [H, E * I], BF16)
    wum = din("wuT", [H, E * I], BF16)
    wdm = din("wdT", [E * I + ISZ, H], BF16)
    wsg = din("wsgT", [H, ISZ], BF16)
    wsu = din("wsuT", [H, ISZ], BF16)
    wgt = din("wgateT", [H, E], F32)
    ogm = din("ogc", [P, KH], BF16)
    ogb = din("ogb", [1, 1], F32)
    bqk = din("bqk", [P, 16], F32)
    bvr = din("bv_row", [1, H], F32R)
    bop = din("bop", [P, KH], F32)
    outm = nc.dram_tensor("out", [H, L], F32, kind="ExternalOutput").ap()

    with tile.TileContext(nc) as tc:
        es = {}  # manually closed long-lived pools

        def open_pool(key, **kw):
            st = ExitStack()
            pool = st.enter_context(tc.tile_pool(name=key, **kw))
            es[key] = st
            return pool

        with ExitStack() as top:
            const = top.enter_context(tc.tile_pool(name="const", bufs=1))

            ident = const.tile([P, P], F32, name="ident")
            make_identity(nc, ident)
            ones_cb = const.tile([P, 1], BF16, name="ones_cb")
            nc.gpsimd.memset(ones_cb[:], 1.0)
            ones_bc_f = const.tile([65, P], F32, name="ones_bc_f")
            nc.gpsimd.memset(ones_bc_f[:], 1.0)
            ones_bc = const.tile([65, P], F32R, name="ones_bc")
            nc.scalar.copy(ones_bc[:], ones_bc_f[:])
            ones_row = ones_bc[0:1, :]
            eps_col = const.tile([P, 1], F32, name="eps_col")
            nc.gpsimd.memset(eps_col[:], EPS)
            tc_sb = const.tile([P, 1], F32, name="tc_sb")
            nc.sync.dma_start(tc_sb[:], tcc[:, :])

            # key-padding masks: maskc[:, kb] = 0 if (kb*128+p) < tc else NEG
            iog = const.tile([P, NT], I32, name="iog")
            nc.gpsimd.iota(iog[:], pattern=[[P, NT]], base=0, channel_multiplier=1)
            iogf = const.tile([P, NT], F32, name="iogf")
            nc.vector.tensor_copy(iogf[:], iog[:])
            mask01 = const.tile([P, NT], F32, name="mask01")
            nc.vector.tensor_scalar(mask01[:], iogf[:], tc_sb[:], None, op0=Alu.is_ge)
            maskc = const.tile([P, NT], F32, name="maskc")
            nc.scalar.mul(maskc[:], mask01[:], NEG)
            # valid[0, n] = 1 if n < tc else 0
            ior = const.tile([1, L], I32, name="ior")
            nc.gpsimd.iota(ior[:], pattern=[[1, L]], base=0, channel_multiplier=0)
            iorf = const.tile([1, L], F32, name="iorf")
            nc.vector.tensor_copy(iorf[:], ior[:])
            valid = const.tile([1, L], F32, name="valid")
            nc.vector.tensor_scalar(valid[:], iorf[:], tc_sb[0:1, :], None, op0=Alu.is_lt)

            bias_p = top.enter_context(tc.tile_pool(name="biasp", bufs=1))
            bqk_sb = bias_p.tile([P, 16], F32, name="bqk")
            nc.sync.dma_start(bqk_sb[:], bqk[:, :])
            bvr_sb = bias_p.tile([1, H], F32R, name="bvr")
            nc.sync.dma_start(bvr_sb[:], bvr[:, :])
            bop_sb = bias_p.tile([P, KH], F32, name="bop")
            nc.sync.dma_start(bop_sb[:], bop[:, :])

            # ---------------- phase A: rms0 + nx ----------------
            nxp = open_pool("nx", bufs=1, side="right")
            NX = [nxp.tile([P, L], F32R, name=f"nx{k}") for k in range(KH)]
            with ExitStack() as ph:
                xp = ph.enter_context(tc.tile_pool(name="xa", bufs=1))
                X = []
                for k in range(KH):
                    t = xp.tile([P, L], F32, name=f"x{k}")
                    nc.sync.dma_start(t[:], xT[k * P:(k + 1) * P, :])
                    X.append(t)
                sq = ph.enter_context(tc.tile_pool(name="sq0", bufs=KH))
                pp = ph.enter_context(tc.tile_pool(name="ps0", bufs=2, space="PSUM"))
                pb = ph.enter_context(tc.tile_pool(name="ps0b", bufs=2, space="PSUM"))
                bc = ph.enter_context(tc.tile_pool(name="bc0", bufs=1))
                xsq = []
                for k in range(KH):
                    t = sq.tile([P, L], BF16, tag="xsq", name="xsq")
                    nc.scalar.activation(t[:], X[k][:], Act.Square)
                    xsq.append(t)
                r0row = bc.tile([1, L], F32, name="r0row")
                sroot = bc.tile([1, L], F32, name="sroot0")
                for j in range(2):
                    ps = pp.tile([1, 512], F32, tag="ss", name="ss")
                    for k in range(KH):
                        nc.tensor.matmul(ps[:], ones_cb[:], xsq[k][:, j * 512:(j + 1) * 512],
                                         start=(k == 0), stop=(k == KH - 1))
                    nc.scalar.activation(sroot[0:1, j * 512:(j + 1) * 512], ps[:],
                                         Act.Sqrt, bias=eps_col[0:1, :], scale=1.0 / H)
                    nc.vector.reciprocal(r0row[0:1, j * 512:(j + 1) * 512],
                                         sroot[0:1, j * 512:(j + 1) * 512])
                r0row_r = bc.tile([1, L], F32R, name="r0row_r")
                nc.scalar.copy(r0row_r[:], r0row[:])
                r0bc = bc.tile([P, L], F32, name="r0bc")
                for j in range(2):
                    psb = pb.tile([P, 512], F32, tag="bc", name="bc")
                    nc.tensor.matmul(psb[:], ones_row[:],
                                     r0row_r[0:1, j * 512:(j + 1) * 512],
                                     start=True, stop=True)
                    nc.scalar.copy(r0bc[:, j * 512:(j + 1) * 512], psb[:])
                for k in range(KH):
                    nc.vector.tensor_mul(NX[k][:], X[k][:], r0bc[:])

            # ---------------- phase B: QKV ----------------
            qkvp = open_pool("qkv", bufs=1)
            Q = [qkvp.tile([P, L], BF16, name=f"q{i}") for i in range(KH)]
            K = [qkvp.tile([P, L], BF16, name=f"k{i}") for i in range(KH)]
            V = [qkvp.tile([P, L], BF16, name=f"v{i}") for i in range(NT)]

            with ExitStack() as ph:
                wp = ph.enter_context(tc.tile_pool(name="wqkv", bufs=1))
                wqk_sb, wv_sb = [], []
                for k in range(KH):
                    t = wp.tile([P, 2 * H], F32R, name=f"wqk_{k}")
                    nc.sync.dma_start(t[:], wqk[k * P:(k + 1) * P, :])
                    wqk_sb.append(t)
                for k in range(KH):
                    t = wp.tile([P, H], F32R, name=f"wv{k}")
                    nc.sync.dma_start(t[:], wvm[k * P:(k + 1) * P, :])
                    wv_sb.append(t)
                pp = ph.enter_context(tc.tile_pool(name="psqk", bufs=4, space="PSUM"))
                for fb in range(16):
                    dst = Q[fb] if fb < KH else K[fb - KH]
                    pts = [pp.tile([P, 512], F32, tag="qk", name="qk") for _ in range(2)]
                    for k in range(KH):
                        for j in range(2):
                            nc.tensor.matmul(
                                pts[j][:],
                                wqk_sb[k][:, fb * P:(fb + 1) * P],
                                NX[k][:, j * 512:(j + 1) * 512],
                                start=(k == 0), stop=(k == KH - 1))
                    for j in range(2):
                        nc.scalar.activation(dst[:, j * 512:(j + 1) * 512], pts[j][:],
                                             Act.Identity, bias=bqk_sb[:, fb:fb + 1])
                for tb in range(NT):
                    pts = [pp.tile([P, 512], F32, tag="v", name="v") for _ in range(2)]
                    for k in range(KH):
                        for j in range(2):
                            nc.tensor.matmul(
                                pts[j][:],
                                NX[k][:, tb * P:(tb + 1) * P],
                                wv_sb[k][:, j * 512:(j + 1) * 512],
                                start=(k == 0), stop=False)
                    for j in range(2):
                        # homogeneous bias row: out += 1 * bv
                        nc.tensor.matmul(pts[j][:], ones_row[:],
                                         bvr_sb[0:1, j * 512:(j + 1) * 512],
                                         start=False, stop=True)
                        nc.vector.tensor_copy(V[tb][:, j * 512:(j + 1) * 512], pts[j][:])
            es["nx"].close()

            # out_proj weights prefetch (DMA overlaps attention)
            wop = open_pool("wo", bufs=1, side="right")
            wo_sb = []
            for k in range(KH):
                t = wop.tile([P, H], F32R, name=f"wo{k}")
                nc.sync.dma_start(t[:], wom[k * P:(k + 1) * P, :])
                wo_sb.append(t)

            # ---------------- phase C: attention ----------------
            ctxp = open_pool("ctx", bufs=1, side="right")
            CTX = [ctxp.tile([P, L], F32R, name=f"ctx{i}") for i in range(KH)]
            with ExitStack() as ph:
                ptp = ph.enter_context(tc.tile_pool(name="pt", bufs=10))
                zp = ph.enter_context(tc.tile_pool(name="zrow", bufs=2))
                zbp = ph.enter_context(tc.tile_pool(name="zbc", bufs=2))
                pa = ph.enter_context(tc.tile_pool(name="psatt", bufs=4, space="PSUM"))
                pz = ph.enter_context(tc.tile_pool(name="psz", bufs=1, space="PSUM"))
                pc = ph.enter_context(tc.tile_pool(name="psctx", bufs=2, space="PSUM"))
                pbb = ph.enter_context(tc.tile_pool(name="psbcz", bufs=1, space="PSUM"))
                for h in range(NH):
                    pts = []
                    for kb in range(NT):
                        pt_t = ptp.tile([P, L], BF16, tag="pt", name="pt")
                        pa_t = [pa.tile([P, 512], F32, tag="att", name="att")
                                for _ in range(2)]
                        for t in range(2):
                            for qh in range(2):
                                nc.tensor.matmul(
                                    pa_t[qh][:],
                                    K[2 * h + t][:, kb * P:(kb + 1) * P],
                                    Q[2 * h + t][:, qh * 512:(qh + 1) * 512],
                                    start=(t == 0), stop=(t == 1))
                        for qh in range(2):
                            nc.scalar.activation(pt_t[:, qh * 512:(qh + 1) * 512],
                                                 pa_t[qh][:],
                                                 Act.Exp, bias=maskc[:, kb:kb + 1],
                                                 scale=INV_SQRT_HD)
                        pts.append(pt_t)
                    zrow = zp.tile([1, L], F32, tag="z", name="z")
                    for qh in range(2):
                        pz_t = pz.tile([1, 512], F32, tag="z", name="zps")
                        for kb in range(NT):
                            nc.tensor.matmul(pz_t[:], ones_cb[:],
                                             pts[kb][:, qh * 512:(qh + 1) * 512],
                                             start=(kb == 0), stop=(kb == NT - 1))
                        nc.vector.reciprocal(zrow[0:1, qh * 512:(qh + 1) * 512], pz_t[:])
                    zrow_r = zp.tile([1, L], F32R, tag="zr", name="zr")
                    nc.scalar.copy(zrow_r[:], zrow[:])
                    zbc = zbp.tile([P, L], F32, tag="zbc", name="zbc")
                    for qh in range(2):
                        pb_t = pbb.tile([P, 512], F32, tag="bcz", name="bcz")
                        nc.tensor.matmul(pb_t[:], ones_row[:],
                                         zrow_r[0:1, qh * 512:(qh + 1) * 512],
                                         start=True, stop=True)
                        nc.scalar.copy(zbc[:, qh * 512:(qh + 1) * 512], pb_t[:])
                    for db in range(ND):
                        pc_t = [pc.tile([P, 512], F32, tag="ctx", name="ctx")
                                for _ in range(2)]
                        for kb in range(NT):
                            for qh in range(2):
                                nc.tensor.matmul(
                                    pc_t[qh][:],
                                    V[kb][:, h * HD + db * P: h * HD + (db + 1) * P],
                                    pts[kb][:, qh * 512:(qh + 1) * 512],
                                    start=(kb == 0), stop=(kb == NT - 1))
                        for qh in range(2):
                            nc.vector.tensor_mul(
                                CTX[2 * h + db][:, qh * 512:(qh + 1) * 512],
                                pc_t[qh][:], zbc[:, qh * 512:(qh + 1) * 512])
            es["qkv"].close()

            # ---------------- phase D: out_proj + residual ----------------
            x1p = open_pool("x1", bufs=1)
            X1 = [x1p.tile([P, L], F32, name=f"x1_{i}") for i in range(KH)]
            with ExitStack() as ph:
                pp = ph.enter_context(tc.tile_pool(name="pso", bufs=4, space="PSUM"))
                xp2 = ph.enter_context(tc.tile_pool(name="xd", bufs=1))
                X = []
                for k in range(KH):
                    t = xp2.tile([P, L], F32, name=f"xd{k}")
                    nc.sync.dma_start(t[:], xT[k * P:(k + 1) * P, :])
                    X.append(t)
                for fb in range(KH):
                    pts = [pp.tile([P, 512], F32, tag="o", name="o") for _ in range(2)]
                    for k in range(KH):
                        for j in range(2):
                            nc.tensor.matmul(
                                pts[j][:],
                                wo_sb[k][:, fb * P:(fb + 1) * P],
                                CTX[k][:, j * 512:(j + 1) * 512],
                                start=(k == 0), stop=(k == KH - 1))
                    for j in range(2):
                        nc.vector.scalar_tensor_tensor(
                            X1[fb][:, j * 512:(j + 1) * 512],
                            pts[j][:], bop_sb[:, fb:fb + 1],
                            X[fb][:, j * 512:(j + 1) * 512],
                            op0=Alu.add, op1=Alu.add)
            es["ctx"].close()
            es["wo"].close()

            # shared-expert weights prefetch (DMA overlaps rms1/gating)
            wexp = open_pool("wexp", bufs=1, side="right")
            wsg_sb, wsu_sb = [], []
            for k in range(KH):
                t = wexp.tile([P, ISZ], BF16, name=f"wsg{k}")
                nc.sync.dma_start(t[:], wsg[k * P:(k + 1) * P, :])
                wsg_sb.append(t)
                t = wexp.tile([P, ISZ], BF16, name=f"wsu{k}")
                nc.sync.dma_start(t[:], wsu[k * P:(k + 1) * P, :])
                wsu_sb.append(t)

            # ---------------- phase E: rms1 + xhat + r_cols ----------------
            xhp = open_pool("xhat", bufs=1, side="right")
            XH = [xhp.tile([P, L], BF16, name=f"xh{k}") for k in range(KH)]
            r_cols = xhp.tile([P, NT], F32, name="r_cols")
            with ExitStack() as ph:
                sq = ph.enter_context(tc.tile_pool(name="sq1", bufs=KH))
                pp = ph.enter_context(tc.tile_pool(name="ps1", bufs=2, space="PSUM"))
                pb = ph.enter_context(tc.tile_pool(name="ps1b", bufs=2, space="PSUM"))
                ptr = ph.enter_context(tc.tile_pool(name="ps1t", bufs=1, space="PSUM"))
                bc = ph.enter_context(tc.tile_pool(name="bc1", bufs=1))
                xsq = []
                for k in range(KH):
                    t = sq.tile([P, L], BF16, tag="x1sq", name="x1sq")
                    nc.scalar.activation(t[:], X1[k][:], Act.Square)
                    xsq.append(t)
                rrow = bc.tile([1, L], F32, name="rrow")
                sroot = bc.tile([1, L], F32, name="sroot1")
                for j in range(2):
                    ps = pp.tile([1, 512], F32, tag="ss", name="ss1")
                    for k in range(KH):
                        nc.tensor.matmul(ps[:], ones_cb[:], xsq[k][:, j * 512:(j + 1) * 512],
                                         start=(k == 0), stop=(k == KH - 1))
                    nc.scalar.activation(sroot[0:1, j * 512:(j + 1) * 512], ps[:],
                                         Act.Sqrt, bias=eps_col[0:1, :], scale=1.0 / H)
                    nc.vector.reciprocal(rrow[0:1, j * 512:(j + 1) * 512],
                                         sroot[0:1, j * 512:(j + 1) * 512])
                rrow_r = bc.tile([1, L], F32R, name="rrow_r")
                nc.scalar.copy(rrow_r[:], rrow[:])
                rbc = bc.tile([P, L], F32, name="rbc")
                for j in range(2):
                    psb = pb.tile([P, 512], F32, tag="bc", name="bc1")
                    nc.tensor.matmul(psb[:], ones_row[:],
                                     rrow_r[0:1, j * 512:(j + 1) * 512],
                                     start=True, stop=True)
                    nc.scalar.copy(rbc[:, j * 512:(j + 1) * 512], psb[:])
                for k in range(KH):
                    nc.vector.tensor_mul(XH[k][:], X1[k][:], rbc[:])
                # r as per-token columns [128, NT] via tiny transposes
                ptt = ptr.tile([P, NT], F32, tag="rt", name="rt")
                for tb in range(NT):
                    nc.tensor.transpose(ptt[:, tb:tb + 1],
                                        rrow[0:1, tb * P:(tb + 1) * P],
                                        ident[0:1, 0:1])
                nc.scalar.copy(r_cols[:], ptt[:])

            # ---------------- phase F: router gating ----------------
            wbcp = open_pool("wbc", bufs=1, side="right")
            WBC = [wbcp.tile([P, L], BF16, name=f"wbc{e}") for e in range(E)]
            wrows = wbcp.tile([E, L], F32R, name="wrows")
            # broadcast-source rows live at base partitions 0/32/64 (matmul rule)
            wrow_t = [wbcp.tile([65, L], F32R, name=f"wrt{i}") for i in range(3)]
            wrow_e = [wrow_t[e // 3][32 * (e % 3):32 * (e % 3) + 1, :] for e in range(E)]
            with ExitStack() as ph:
                wp = ph.enter_context(tc.tile_pool(name="wgate", bufs=1))
                gp = ph.enter_context(tc.tile_pool(name="gating", bufs=4))
                pg = ph.enter_context(tc.tile_pool(name="psg", bufs=4, space="PSUM"))
                pt_ = ph.enter_context(tc.tile_pool(name="psgt", bufs=2, space="PSUM"))
                pwb = ph.enter_context(tc.tile_pool(name="pswb", bufs=2, space="PSUM"))
                wgt_sb = []
                for k in range(KH):
                    t = wp.tile([P, E], F32, name=f"wgt{k}")
                    nc.sync.dma_start(t[:], wgt[k * P:(k + 1) * P, :])
                    wgt_sb.append(t)
                for tb in range(NT):
                    pg_t = pg.tile([P, E], F32, tag="g", name="g")
                    for k in range(KH):
                        nc.tensor.matmul(pg_t[:], X1[k][:, tb * P:(tb + 1) * P], wgt_sb[k][:],
                                         start=(k == 0), stop=(k == KH - 1))
                    s_t = gp.tile([P, E], F32, tag="s", name="s")
                    nc.scalar.activation(s_t[:], pg_t[:], Act.Exp,
                                         scale=r_cols[:, tb:tb + 1])
                    m1 = gp.tile([P, 1], F32, tag="m1", name="m1")
                    nc.vector.reduce_max(m1[:], s_t[:], axis=AX.X)
                    ml = gp.tile([P, E], F32, tag="ml", name="ml")
                    nc.vector.tensor_scalar(ml[:], s_t[:], m1[:], None, op0=Alu.is_lt)
                    s2 = gp.tile([P, E], F32, tag="s2", name="s2")
                    nc.vector.tensor_mul(s2[:], s_t[:], ml[:])
                    m2 = gp.tile([P, 1], F32, tag="m2", name="m2")
                    nc.vector.reduce_max(m2[:], s2[:], axis=AX.X)
                    keep = gp.tile([P, E], F32, tag="keep", name="keep")
                    nc.vector.tensor_scalar(keep[:], s_t[:], m2[:], None, op0=Alu.is_ge)
                    ssum = gp.tile([P, 1], F32, tag="ssum", name="ssum")
                    nc.vector.tensor_add(ssum[:], m1[:], m2[:])
                    srec = gp.tile([P, 1], F32, tag="srec", name="srec")
                    nc.vector.reciprocal(srec[:], ssum[:])
                    wt = gp.tile([P, E], F32, tag="wt", name="wt")
                    nc.vector.scalar_tensor_tensor(wt[:], s_t[:], srec[:], keep[:],
                                                   op0=Alu.mult, op1=Alu.mult)
                    pt_t = pt_.tile([E, P], F32, tag="wtT", name="wtT")
                    nc.tensor.transpose(pt_t[:], wt[:], ident[:])
                    nc.scalar.copy(wrows[:, tb * P:(tb + 1) * P], pt_t[:])
                for e in range(E):
                    nc.sync.dma_start(wrow_e[e][:], wrows[e:e + 1, :])
                for e in range(E):
                    for j in range(2):
                        pw_t = pwb.tile([P, 512], F32, tag="wbc", name="wbcp")
                        base = 32 * (e % 3)
                        nc.tensor.matmul(pw_t[:], ones_bc[base:base + 1, :],
                                         wrow_e[e][0:1, j * 512:(j + 1) * 512],
                                         start=True, stop=True)
                        nc.scalar.copy(WBC[e][:, j * 512:(j + 1) * 512], pw_t[:])
            es["x1"].close()

            # ---------------- phase G: routed expert gate/up ----------------
            ap_ = open_pool("acts", bufs=1)
            A = [ap_.tile([P, L], BF16, name=f"a{i}") for i in range(2 * E)]
            ASH = [ap_.tile([P, L], BF16, name=f"ash{i}") for i in range(ISZ // P)]
            with ExitStack() as ph:
                tmp = ph.enter_context(tc.tile_pool(name="tmpgu", bufs=2))
                wst = ph.enter_context(tc.tile_pool(name="wgus", bufs=24))
                pp = ph.enter_context(tc.tile_pool(name="psgu", bufs=8, space="PSUM"))
                for fb in range(2 * E):
                    e = fb // 2
                    wgf = []
                    for k in range(KH):
                        t = wst.tile([P, P], BF16, tag="wgs", name="wgs")
                        nc.sync.dma_start(t[:], wgm[k * P:(k + 1) * P, fb * P:(fb + 1) * P])
                        wgf.append(t)
                    wuf = []
                    for k in range(KH):
                        t = wst.tile([P, P], BF16, tag="wus", name="wus")
                        nc.sync.dma_start(t[:], wum[k * P:(k + 1) * P, fb * P:(fb + 1) * P])
                        wuf.append(t)
                    pg_ = [pp.tile([P, 512], F32, tag="gu", name="gu") for _ in range(2)]
                    for k in range(KH):
                        for j in range(2):
                            nc.tensor.matmul(pg_[j][:], wgf[k][:],
                                             XH[k][:, j * 512:(j + 1) * 512],
                                             start=(k == 0), stop=(k == KH - 1))
                    sgm = tmp.tile([P, L], BF16, tag="sgm", name="sgm")
                    for j in range(2):
                        nc.scalar.activation(sgm[:, j * 512:(j + 1) * 512], pg_[j][:],
                                             Act.Sigmoid)
                    sg = tmp.tile([P, L], BF16, tag="sg", name="sg")
                    for j in range(2):
                        nc.vector.tensor_mul(sg[:, j * 512:(j + 1) * 512], pg_[j][:],
                                             sgm[:, j * 512:(j + 1) * 512])
                    pu_ = [pp.tile([P, 512], F32, tag="gu", name="gu") for _ in range(2)]
                    for k in range(KH):
                        for j in range(2):
                            nc.tensor.matmul(pu_[j][:], wuf[k][:],
                                             XH[k][:, j * 512:(j + 1) * 512],
                                             start=(k == 0), stop=(k == KH - 1))
                    ta = tmp.tile([P, L], BF16, tag="ta", name="ta")
                    for j in range(2):
                        nc.vector.tensor_mul(ta[:, j * 512:(j + 1) * 512], pu_[j][:],
                                             sg[:, j * 512:(j + 1) * 512])
                    nc.vector.tensor_mul(A[fb][:], ta[:], WBC[e][:])
            es["wbc"].close()

            # down-proj weights prefetch (DMA overlaps shared expert phase)
            wdp = open_pool("wd", bufs=1)
            NKD = 2 * E + ISZ // P  # 20
            wd_sb = []
            for k in range(NKD):
                t = wdp.tile([P, H], BF16, name=f"wd{k}")
                nc.sync.dma_start(t[:], wdm[k * P:(k + 1) * P, :])
                wd_sb.append(t)

            # ---------------- phase H: shared expert gate/up ----------------
            with ExitStack() as ph:
                tmp = ph.enter_context(tc.tile_pool(name="tmpsgu", bufs=2))
                pp = ph.enter_context(tc.tile_pool(name="pssgu", bufs=8, space="PSUM"))
                for fb in range(ISZ // P):
                    pg_ = [pp.tile([P, 512], F32, tag="sgu", name="sgu") for _ in range(2)]
                    for k in range(KH):
                        for j in range(2):
                            nc.tensor.matmul(pg_[j][:], wsg_sb[k][:, fb * P:(fb + 1) * P],
                                             XH[k][:, j * 512:(j + 1) * 512],
                                             start=(k == 0), stop=(k == KH - 1))
                    sgm = tmp.tile([P, L], BF16, tag="ssgm", name="ssgm")
                    for j in range(2):
                        nc.scalar.activation(sgm[:, j * 512:(j + 1) * 512], pg_[j][:],
                                             Act.Sigmoid)
                    sg = tmp.tile([P, L], BF16, tag="ssg", name="ssg")
                    for j in range(2):
                        nc.vector.tensor_mul(sg[:, j * 512:(j + 1) * 512], pg_[j][:],
                                             sgm[:, j * 512:(j + 1) * 512])
                    pu_ = [pp.tile([P, 512], F32, tag="sgu", name="sgu") for _ in range(2)]
                    for k in range(KH):
                        for j in range(2):
                            nc.tensor.matmul(pu_[j][:], wsu_sb[k][:, fb * P:(fb + 1) * P],
                                             XH[k][:, j * 512:(j + 1) * 512],
                                             start=(k == 0), stop=(k == KH - 1))
                    for j in range(2):
                        nc.vector.tensor_mul(ASH[fb][:, j * 512:(j + 1) * 512], pu_[j][:],
                                             sg[:, j * 512:(j + 1) * 512])
            es["xhat"].close()
            es["wexp"].close()

            # ---------------- phase I: down proj (routed + shared fused) ----------------
            yp = open_pool("y", bufs=1, side="right")
            Y = [yp.tile([P, L], F32, name=f"y{i}") for i in range(KH)]
            YB = [yp.tile([P, L], BF16, name=f"yb{i}") for i in range(KH)]
            AALL = A + ASH
            with ExitStack() as ph:
                pp = ph.enter_context(tc.tile_pool(name="psd", bufs=6, space="PSUM"))
                for hb in range(KH):
                    pts = [pp.tile([P, 512], F32, tag="y", name="yps") for _ in range(2)]
                    for k in range(NKD):
                        for j in range(2):
                            nc.tensor.matmul(pts[j][:], wd_sb[k][:, hb * P:(hb + 1) * P],
                                             AALL[k][:, j * 512:(j + 1) * 512],
                                             start=(k == 0), stop=(k == NKD - 1))
                    for j in range(2):
                        nc.scalar.copy(Y[hb][:, j * 512:(j + 1) * 512], pts[j][:])
                        nc.vector.tensor_copy(YB[hb][:, j * 512:(j + 1) * 512], pts[j][:])
            es["wd"].close()
            es["acts"].close()

            # ---------------- phase J: output gate + final mask ----------------
            with ExitStack() as ph:
                wp = ph.enter_context(tc.tile_pool(name="wog", bufs=1))
                fr = ph.enter_context(tc.tile_pool(name="final", bufs=1))
                op_ = ph.enter_context(tc.tile_pool(name="outp", bufs=3))
                pg = ph.enter_context(tc.tile_pool(name="psog", bufs=2, space="PSUM"))
                pbf = ph.enter_context(tc.tile_pool(name="psfin", bufs=1, space="PSUM"))
                ogc_sb = wp.tile([P, KH], BF16, name="ogc")
                nc.sync.dma_start(ogc_sb[:], ogm[:, :])
                ogb_sb = wp.tile([1, 1], F32, name="ogb")
                nc.sync.dma_start(ogb_sb[:], ogb[:, :])
                sigrow = fr.tile([1, L], F32, name="sigrow")
                for j in range(2):
                    pg_t = pg.tile([1, 512], F32, tag="og", name="og")
                    for k in range(KH):
                        nc.tensor.matmul(pg_t[:], ogc_sb[:, k:k + 1],
                                         YB[k][:, j * 512:(j + 1) * 512],
                                         start=(k == 0), stop=(k == KH - 1))
                    nc.scalar.activation(sigrow[0:1, j * 512:(j + 1) * 512], pg_t[:],
                                         Act.Sigmoid, bias=ogb_sb[0:1, :])
                svrow = fr.tile([1, L], F32R, name="svrow")
                nc.vector.tensor_mul(svrow[:], sigrow[:], valid[:])
                svb = fr.tile([P, L], F32, name="svb")
                for j in range(2):
                    pb_t = pbf.tile([P, 512], F32, tag="fin", name="fin")
                    nc.tensor.matmul(pb_t[:], ones_row[:],
                                     svrow[0:1, j * 512:(j + 1) * 512],
                                     start=True, stop=True)
                    nc.scalar.copy(svb[:, j * 512:(j + 1) * 512], pb_t[:])
                for hb in range(KH):
                    ot = op_.tile([P, L], F32, tag="ot", name="ot")
                    nc.vector.tensor_mul(ot[:], Y[hb][:], svb[:])
                    nc.sync.dma_start(outm[hb * P:(hb + 1) * P, :], ot[:])
            es["y"].close()

    nc.compile()
    return nc


_CACHE = {}


def _get_program():
    if "nc" not in _CACHE:
        _CACHE["nc"] = build()
    return _CACHE["nc"]


def _prep_inputs(inputs):
    f32 = np.float32
    bf = ml_dtypes.bfloat16
    g = lambda k: np.asarray(inputs[k]).astype(f32)

    hs = g("hidden_states")
    tcs = np.asarray(inputs["true_counts"]).astype(np.int64).reshape(B)
    cnw, gnw, snw = g("context_norm_w"), g("gate_norm_w"), g("shared_norm_w")
    ipw, ipb = g("in_proj_w"), g("in_proj_b")
    opw, opb = g("out_proj_w"), g("out_proj_b")
    gw = g("gate_w")
    enw = g("expert_norm_w")
    egw, euw, edw = g("expert_gate_w"), g("expert_up_w"), g("expert_down_w")
    sgw, suw, sdw = g("shared_gate_w"), g("shared_up_w"), g("shared_down_w")
    ogw, ogb_ = g("out_gate_w"), g("out_gate_b")

    shared = {
        "wqkT": np.ascontiguousarray((ipw[:2 * H] * cnw[None, :]).T),
        "wvT": np.ascontiguousarray((ipw[2 * H:] * cnw[None, :]).T),
        "woT": np.ascontiguousarray(opw.T),
        "wgT": np.ascontiguousarray((egw * enw[:, None, :]).reshape(E * I, H).T.astype(bf)),
        "wuT": np.ascontiguousarray((euw * enw[:, None, :]).reshape(E * I, H).T.astype(bf)),
        "wdT": np.ascontiguousarray(np.concatenate(
            [edw.transpose(0, 2, 1).reshape(E * I, H), sdw.T], axis=0).astype(bf)),
        "wsgT": np.ascontiguousarray((sgw * snw[None, :]).T.astype(bf)),
        "wsuT": np.ascontiguousarray((suw * snw[None, :]).T.astype(bf)),
        "wgateT": np.ascontiguousarray((gw * gnw[None, :]).T),
        "ogc": np.ascontiguousarray(ogw.reshape(KH, P).T.astype(bf)),
        "ogb": ogb_.reshape(1, 1),
        "bqk": np.ascontiguousarray(ipb[:2 * H].reshape(16, P).T),
        "bv_row": np.ascontiguousarray(ipb[2 * H:].reshape(1, H)),
        "bop": np.ascontiguousarray(opb.reshape(KH, P).T),
    }
    in_maps = []
    for b in range(B):
        m = dict(shared)
        m["x_t"] = np.ascontiguousarray(hs[b].T)
        m["tc_col"] = np.full((P, 1), float(tcs[b]), f32)
        in_maps.append(m)
    return in_maps


LAST_RESULT = None


def _run(inputs, **kw):
    global LAST_RESULT
    nc = _get_program()
    in_maps = _prep_inputs(inputs)
    res = run_bass_kernel_spmd(nc, in_maps, core_ids=list(range(B)), **kw)
    LAST_RESULT = res
    out = np.stack([res.results[b]["out"].T for b in range(B)])
    return np.ascontiguousarray(out.astype(np.float32))


def kernel(**inputs):
    return _run(inputs)



# revision 4
# speedup vs baseline: 5.1311x; 5.1311x over previous
"""DeepseekMoE block (attention + top-2 routed MoE + shared expert) on 8 TRN2
NeuronCores, data-parallel over the batch dimension (B=8 -> one batch per core).

End-to-end latency here is dominated by host<->device transfer through the
tunnel, so the kernel is organized to minimize shipped bytes:
  - Weights are shipped SHARDED 1/8th per core and reassembled on-device with
    HBM->HBM AllGather collectives (each weight byte crosses the link once
    instead of 8x).
  - The attention chain (x, qkv/out_proj weights, scores, ctx) runs in fp16
    (half the bytes of fp32 at ~8x less noise than bf16 -- the router's top-2
    selection is sensitive to noise in x + attn_out). Expert FFNs run in bf16.
    Router logits stay fp32.
  - The output is returned as fp16 [H, LP] and cast/padded on host.
  - The token dimension is truncated to LP = ceil(max(true_counts)/128)*128;
    padded tokens are masked as attention keys and zeroed at the output, so
    they cannot influence valid outputs.

Layout strategy per core (LP tokens, H=1024 hidden): activations live in
"F-layout" [feature-on-partitions, tokens-on-free]; per-token scalars are
produced as [1, LP] rows and broadcast across partitions with K=1 rank-1
matmuls on the TensorEngine; attention is computed transposed (attT[k, q]) so
the key-padding mask and exp() fold into one scalar-engine activation.
"""

import numpy as np
import ml_dtypes
from contextlib import ExitStack

import concourse.bass as bass
import concourse.mybir as mybir
import concourse.tile as tile
from concourse import bacc
from concourse.bass_utils import run_bass_kernel_spmd
from concourse.masks import make_identity

B, L, H = 8, 1024, 1024
E, I, NH, HD = 8, 256, 4, 256
ISZ = 512
P = 128
KH = H // P      # hidden slabs (8)
EPS = 1e-6
NEG = -30000.0
INV_SQRT_HD = float(1.0 / np.sqrt(HD))
NCORES = 8
NKD = 2 * E + ISZ // P   # down-proj K slabs (20)
WD_ROWS = E * I + ISZ    # 4608

DT = mybir.dt
F32, BF16, F16, I32 = DT.float32, DT.bfloat16, DT.float16, DT.int32
F32R = DT.float32r
Alu = mybir.AluOpType
Act = mybir.ActivationFunctionType
AX = mybir.AxisListType


def build(NT):
    LP = NT * P
    # token-dim chunks (moving free dim <= 512, one PSUM bank each)
    JT = [(0, LP)] if LP <= 512 else [(0, 512), (512, LP - 512)]
    JH = [(0, 512), (512, 512)]  # hidden-dim chunks (always H=1024)

    nc = bacc.Bacc("TRN2", target_bir_lowering=False, debug=False,
                   num_devices=NCORES)

    def din(name, shape, dt):
        return nc.dram_tensor(name, shape, dt, kind="ExternalInput").ap()

    xT = din("x_t", [H, LP], F16)
    tcc = din("tc_col", [P, 1], F32)
    wattn_s = din("wattn_sh", [P, 4 * H], F16)     # [wq|wk|wv|wo] row-shard
    wgu_s = din("wgu_sh", [P, 2 * E * I], BF16)    # [wg|wu] row-shard
    wd_s = din("wd_sh", [WD_ROWS // NCORES, H], BF16)
    wsgu_s = din("wsgu_sh", [P, 2 * ISZ], BF16)    # [wsg|wsu] row-shard
    wgt = din("wgateT", [H, E], F32)
    ogm = din("ogc", [P, KH], BF16)
    ogb = din("ogb", [1, 1], F32)
    bqk = din("bqk", [P, 16], F32)
    bvr = din("bv_row", [1, H], F16)
    bop = din("bop", [P, KH], F32)
    outm = nc.dram_tensor("out", [H, LP], F16, kind="ExternalOutput").ap()

    RG = [list(range(NCORES))]

    with tile.TileContext(nc) as tc:
        es = {}  # manually closed long-lived pools

        def open_pool(key, **kw):
            st = ExitStack()
            pool = st.enter_context(tc.tile_pool(name=key, **kw))
            es[key] = st
            return pool

        with ExitStack() as top:
            const = top.enter_context(tc.tile_pool(name="const", bufs=1))

            ident = const.tile([P, P], F32, name="ident")
            make_identity(nc, ident)
            ones_cb = const.tile([P, 1], BF16, name="ones_cb")
            nc.gpsimd.memset(ones_cb[:], 1.0)
            ones_ch = const.tile([P, 1], F16, name="ones_ch")
            nc.gpsimd.memset(ones_ch[:], 1.0)
            ones_bc_f = const.tile([65, P], F32, name="ones_bc_f")
            nc.gpsimd.memset(ones_bc_f[:], 1.0)
            ones_bc = const.tile([65, P], F32R, name="ones_bc")
            nc.scalar.copy(ones_bc[:], ones_bc_f[:])
            ones_row = ones_bc[0:1, :]
            ones_row_h = const.tile([1, P], F16, name="ones_row_h")
            nc.gpsimd.memset(ones_row_h[:], 1.0)
            eps_col = const.tile([P, 1], F32, name="eps_col")
            nc.gpsimd.memset(eps_col[:], EPS)
            tc_sb = const.tile([P, 1], F32, name="tc_sb")
            nc.sync.dma_start(tc_sb[:], tcc[:, :])

            # key-padding masks: maskc[:, kb] = 0 if (kb*128+p) < tc else NEG
            iog = const.tile([P, NT], I32, name="iog")
            nc.gpsimd.iota(iog[:], pattern=[[P, NT]], base=0, channel_multiplier=1)
            iogf = const.tile([P, NT], F32, name="iogf")
            nc.vector.tensor_copy(iogf[:], iog[:])
            mask01 = const.tile([P, NT], F32, name="mask01")
            nc.vector.tensor_scalar(mask01[:], iogf[:], tc_sb[:], None, op0=Alu.is_ge)
            maskc = const.tile([P, NT], F32, name="maskc")
            nc.scalar.mul(maskc[:], mask01[:], NEG)
            # valid[0, n] = 1 if n < tc else 0
            ior = const.tile([1, LP], I32, name="ior")
            nc.gpsimd.iota(ior[:], pattern=[[1, LP]], base=0, channel_multiplier=0)
            iorf = const.tile([1, LP], F32, name="iorf")
            nc.vector.tensor_copy(iorf[:], ior[:])
            valid = const.tile([1, LP], F32, name="valid")
            nc.vector.tensor_scalar(valid[:], iorf[:], tc_sb[0:1, :], None, op0=Alu.is_lt)

            # ---- weight allgather: shard -> bounce -> gathered (HBM) ----
            # gpsimd runs these after the const memset/iota above; weight
            # loads (sync engine DMAs) wait on the matching gather, so the
            # gathers overlap phase A compute.
            dramp = top.enter_context(tc.tile_pool(name="dramw", bufs=1,
                                                   space="DRAM"))
            wattn_b = dramp.tile([P, 4 * H], F16, name="wattn_b")
            wgu_b = dramp.tile([P, 2 * E * I], BF16, name="wgu_b")
            wsgu_b = dramp.tile([P, 2 * ISZ], BF16, name="wsgu_b")
            wd_b = dramp.tile([WD_ROWS // NCORES, H], BF16, name="wd_b")
            wattn_g = dramp.tile([H, 4 * H], F16, name="wattn_g")
            wgu_g = dramp.tile([H, 2 * E * I], BF16, name="wgu_g")
            wsgu_g = dramp.tile([H, 2 * ISZ], BF16, name="wsgu_g")
            wd_g = dramp.tile([WD_ROWS, H], BF16, name="wd_g")
            for src, bnc, dst in ((wattn_s, wattn_b, wattn_g),
                                  (wgu_s, wgu_b, wgu_g),
                                  (wsgu_s, wsgu_b, wsgu_g),
                                  (wd_s, wd_b, wd_g)):
                nc.gpsimd.dma_start(bnc[:], src[:, :])
                nc.gpsimd.collective_compute(
                    "AllGather", Alu.bypass, replica_groups=RG,
                    ins=[bnc[:].opt()], outs=[dst[:].opt()])

            bias_p = top.enter_context(tc.tile_pool(name="biasp", bufs=1))
            bqk_sb = bias_p.tile([P, 16], F32, name="bqk")
            nc.sync.dma_start(bqk_sb[:], bqk[:, :])
            bvr_sb = bias_p.tile([1, H], F16, name="bvr")
            nc.sync.dma_start(bvr_sb[:], bvr[:, :])
            bop_sb = bias_p.tile([P, KH], F32, name="bop")
            nc.sync.dma_start(bop_sb[:], bop[:, :])

            # ---------------- phase A: rms0 + nx ----------------
            nxp = open_pool("nx", bufs=1, side="right")
            NX = [nxp.tile([P, LP], F16, name=f"nx{k}") for k in range(KH)]
            with ExitStack() as ph:
                xp = ph.enter_context(tc.tile_pool(name="xa", bufs=1))
                X = []
                for k in range(KH):
                    t = xp.tile([P, LP], F16, name=f"x{k}")
                    nc.sync.dma_start(t[:], xT[k * P:(k + 1) * P, :])
                    X.append(t)
                sq = ph.enter_context(tc.tile_pool(name="sq0", bufs=KH))
                pp = ph.enter_context(tc.tile_pool(name="ps0", bufs=2, space="PSUM"))
                pb = ph.enter_context(tc.tile_pool(name="ps0b", bufs=2, space="PSUM"))
                bc = ph.enter_context(tc.tile_pool(name="bc0", bufs=1))
                xsq = []
                for k in range(KH):
                    t = sq.tile([P, LP], BF16, tag="xsq", name="xsq")
                    nc.scalar.activation(t[:], X[k][:], Act.Square)
                    xsq.append(t)
                r0row = bc.tile([1, LP], F32, name="r0row")
                sroot = bc.tile([1, LP], F32, name="sroot0")
                for jo, jw in JT:
                    ps = pp.tile([1, 512], F32, tag="ss", name="ss")
                    for k in range(KH):
                        nc.tensor.matmul(ps[:, :jw], ones_cb[:], xsq[k][:, jo:jo + jw],
                                         start=(k == 0), stop=(k == KH - 1))
                    nc.scalar.activation(sroot[0:1, jo:jo + jw], ps[:, :jw],
                                         Act.Sqrt, bias=eps_col[0:1, :], scale=1.0 / H)
                    nc.vector.reciprocal(r0row[0:1, jo:jo + jw],
                                         sroot[0:1, jo:jo + jw])
                r0row_r = bc.tile([1, LP], F32R, name="r0row_r")
                nc.scalar.copy(r0row_r[:], r0row[:])
                r0bc = bc.tile([P, LP], F32, name="r0bc")
                for jo, jw in JT:
                    psb = pb.tile([P, 512], F32, tag="bc", name="bc")
                    nc.tensor.matmul(psb[:, :jw], ones_row[:],
                                     r0row_r[0:1, jo:jo + jw],
                                     start=True, stop=True)
                    nc.scalar.copy(r0bc[:, jo:jo + jw], psb[:, :jw])
                for k in range(KH):
                    nc.vector.tensor_mul(NX[k][:], X[k][:], r0bc[:])

            # ---------------- phase B: QKV ----------------
            qkvp = open_pool("qkv", bufs=1)
            Q = [qkvp.tile([P, LP], F16, name=f"q{i}") for i in range(KH)]
            K = [qkvp.tile([P, LP], F16, name=f"k{i}") for i in range(KH)]
            V = [qkvp.tile([P, H], F16, name=f"v{i}") for i in range(NT)]

            with ExitStack() as ph:
                wp = ph.enter_context(tc.tile_pool(name="wqkv", bufs=1))
                wqk_sb, wv_sb = [], []
                for k in range(KH):
                    t = wp.tile([P, 2 * H], F16, name=f"wqk_{k}")
                    nc.sync.dma_start(t[:], wattn_g[k * P:(k + 1) * P, 0:2 * H])
                    wqk_sb.append(t)
                for k in range(KH):
                    t = wp.tile([P, H], F16, name=f"wv{k}")
                    nc.sync.dma_start(t[:], wattn_g[k * P:(k + 1) * P, 2 * H:3 * H])
                    wv_sb.append(t)
                pp = ph.enter_context(tc.tile_pool(name="psqk", bufs=4, space="PSUM"))
                for fb in range(16):
                    dst = Q[fb] if fb < KH else K[fb - KH]
                    pts = [pp.tile([P, 512], F32, tag="qk", name="qk") for _ in JT]
                    for k in range(KH):
                        for j, (jo, jw) in enumerate(JT):
                            nc.tensor.matmul(
                                pts[j][:, :jw],
                                wqk_sb[k][:, fb * P:(fb + 1) * P],
                                NX[k][:, jo:jo + jw],
                                start=(k == 0), stop=(k == KH - 1))
                    for j, (jo, jw) in enumerate(JT):
                        nc.scalar.activation(dst[:, jo:jo + jw], pts[j][:, :jw],
                                             Act.Identity, bias=bqk_sb[:, fb:fb + 1])
                for tb in range(NT):
                    pts = [pp.tile([P, 512], F32, tag="v", name="v") for _ in JH]
                    for k in range(KH):
                        for j, (jo, jw) in enumerate(JH):
                            nc.tensor.matmul(
                                pts[j][:, :jw],
                                NX[k][:, tb * P:(tb + 1) * P],
                                wv_sb[k][:, jo:jo + jw],
                                start=(k == 0), stop=False)
                    for j, (jo, jw) in enumerate(JH):
                        # homogeneous bias row: out += 1 * bv
                        nc.tensor.matmul(pts[j][:, :jw], ones_row_h[:],
                                         bvr_sb[0:1, jo:jo + jw],
                                         start=False, stop=True)
                        nc.vector.tensor_copy(V[tb][:, jo:jo + jw], pts[j][:, :jw])
            es["nx"].close()

            # out_proj weights prefetch (DMA overlaps attention)
            wop = open_pool("wo", bufs=1, side="right")
            wo_sb = []
            for k in range(KH):
                t = wop.tile([P, H], F16, name=f"wo{k}")
                nc.sync.dma_start(t[:], wattn_g[k * P:(k + 1) * P, 3 * H:4 * H])
                wo_sb.append(t)

            # ---------------- phase C: attention ----------------
            ctxp = open_pool("ctx", bufs=1, side="right")
            CTX = [ctxp.tile([P, LP], F16, name=f"ctx{i}") for i in range(KH)]
            with ExitStack() as ph:
                ptp = ph.enter_context(tc.tile_pool(name="pt", bufs=NT + 2))
                zp = ph.enter_context(tc.tile_pool(name="zrow", bufs=2))
                zbp = ph.enter_context(tc.tile_pool(name="zbc", bufs=2))
                pa = ph.enter_context(tc.tile_pool(name="psatt", bufs=4, space="PSUM"))
                pz = ph.enter_context(tc.tile_pool(name="psz", bufs=1, space="PSUM"))
                pc = ph.enter_context(tc.tile_pool(name="psctx", bufs=2, space="PSUM"))
                pbb = ph.enter_context(tc.tile_pool(name="psbcz", bufs=1, space="PSUM"))
                for h in range(NH):
                    pts = []
                    for kb in range(NT):
                        pt_t = ptp.tile([P, LP], F16, tag="pt", name="pt")
                        pa_t = [pa.tile([P, 512], F32, tag="att", name="att")
                                for _ in JT]
                        for t in range(2):
                            for j, (jo, jw) in enumerate(JT):
                                nc.tensor.matmul(
                                    pa_t[j][:, :jw],
                                    K[2 * h + t][:, kb * P:(kb + 1) * P],
                                    Q[2 * h + t][:, jo:jo + jw],
                                    start=(t == 0), stop=(t == 1))
                        for j, (jo, jw) in enumerate(JT):
                            nc.scalar.activation(pt_t[:, jo:jo + jw],
                                                 pa_t[j][:, :jw],
                                                 Act.Exp, bias=maskc[:, kb:kb + 1],
                                                 scale=INV_SQRT_HD)
                        pts.append(pt_t)
                    zrow = zp.tile([1, LP], F32, tag="z", name="z")
                    for jo, jw in JT:
                        pz_t = pz.tile([1, 512], F32, tag="z", name="zps")
                        for kb in range(NT):
                            nc.tensor.matmul(pz_t[:, :jw], ones_ch[:],
                                             pts[kb][:, jo:jo + jw],
                                             start=(kb == 0), stop=(kb == NT - 1))
                        nc.vector.reciprocal(zrow[0:1, jo:jo + jw], pz_t[:, :jw])
                    zrow_r = zp.tile([1, LP], F32R, tag="zr", name="zr")
                    nc.scalar.copy(zrow_r[:], zrow[:])
                    zbc = zbp.tile([P, LP], F32, tag="zbc", name="zbc")
                    for jo, jw in JT:
                        pb_t = pbb.tile([P, 512], F32, tag="bcz", name="bcz")
                        nc.tensor.matmul(pb_t[:, :jw], ones_row[:],
                                         zrow_r[0:1, jo:jo + jw],
                                         start=True, stop=True)
                        nc.scalar.copy(zbc[:, jo:jo + jw], pb_t[:, :jw])
                    for db in range(2):
                        pc_t = [pc.tile([P, 512], F32, tag="ctx", name="ctx")
                                for _ in JT]
                        for kb in range(NT):
                            for j, (jo, jw) in enumerate(JT):
                                nc.tensor.matmul(
                                    pc_t[j][:, :jw],
                                    V[kb][:, h * HD + db * P: h * HD + (db + 1) * P],
                                    pts[kb][:, jo:jo + jw],
                                    start=(kb == 0), stop=(kb == NT - 1))
                        for j, (jo, jw) in enumerate(JT):
                            nc.vector.tensor_mul(
                                CTX[2 * h + db][:, jo:jo + jw],
                                pc_t[j][:, :jw], zbc[:, jo:jo + jw])
            es["qkv"].close()

            # ---------------- phase D: out_proj + residual ----------------
            x1p = open_pool("x1", bufs=1)
            X1 = [x1p.tile([P, LP], F32, name=f"x1_{i}") for i in range(KH)]
            with ExitStack() as ph:
                pp = ph.enter_context(tc.tile_pool(name="pso", bufs=4, space="PSUM"))
                xp2 = ph.enter_context(tc.tile_pool(name="xd", bufs=1))
                X = []
                for k in range(KH):
                    t = xp2.tile([P, LP], F16, name=f"xd{k}")
                    nc.sync.dma_start(t[:], xT[k * P:(k + 1) * P, :])
                    X.append(t)
                for fb in range(KH):
                    pts = [pp.tile([P, 512], F32, tag="o", name="o") for _ in JT]
                    for k in range(KH):
                        for j, (jo, jw) in enumerate(JT):
                            nc.tensor.matmul(
                                pts[j][:, :jw],
                                wo_sb[k][:, fb * P:(fb + 1) * P],
                                CTX[k][:, jo:jo + jw],
                                start=(k == 0), stop=(k == KH - 1))
                    for j, (jo, jw) in enumerate(JT):
                        nc.vector.scalar_tensor_tensor(
                            X1[fb][:, jo:jo + jw],
                            pts[j][:, :jw], bop_sb[:, fb:fb + 1],
                            X[fb][:, jo:jo + jw],
                            op0=Alu.add, op1=Alu.add)
            es["ctx"].close()
            es["wo"].close()

            # shared-expert weights prefetch (DMA overlaps rms1/gating)
            wexp = open_pool("wexp", bufs=1, side="right")
            wsg_sb, wsu_sb = [], []
            for k in range(KH):
                t = wexp.tile([P, ISZ], BF16, name=f"wsg{k}")
                nc.sync.dma_start(t[:], wsgu_g[k * P:(k + 1) * P, 0:ISZ])
                wsg_sb.append(t)
                t = wexp.tile([P, ISZ], BF16, name=f"wsu{k}")
                nc.sync.dma_start(t[:], wsgu_g[k * P:(k + 1) * P, ISZ:2 * ISZ])
                wsu_sb.append(t)

            # ---------------- phase E: rms1 + xhat + r_cols ----------------
            xhp = open_pool("xhat", bufs=1, side="right")
            XH = [xhp.tile([P, LP], BF16, name=f"xh{k}") for k in range(KH)]
            r_cols = xhp.tile([P, NT], F32, name="r_cols")
            with ExitStack() as ph:
                sq = ph.enter_context(tc.tile_pool(name="sq1", bufs=KH))
                pp = ph.enter_context(tc.tile_pool(name="ps1", bufs=2, space="PSUM"))
                pb = ph.enter_context(tc.tile_pool(name="ps1b", bufs=2, space="PSUM"))
                ptr = ph.enter_context(tc.tile_pool(name="ps1t", bufs=1, space="PSUM"))
                bc = ph.enter_context(tc.tile_pool(name="bc1", bufs=1))
                xsq = []
                for k in range(KH):
                    t = sq.tile([P, LP], BF16, tag="x1sq", name="x1sq")
                    nc.scalar.activation(t[:], X1[k][:], Act.Square)
                    xsq.append(t)
                rrow = bc.tile([1, LP], F32, name="rrow")
                sroot = bc.tile([1, LP], F32, name="sroot1")
                for jo, jw in JT:
                    ps = pp.tile([1, 512], F32, tag="ss", name="ss1")
                    for k in range(KH):
                        nc.tensor.matmul(ps[:, :jw], ones_cb[:], xsq[k][:, jo:jo + jw],
                                         start=(k == 0), stop=(k == KH - 1))
                    nc.scalar.activation(sroot[0:1, jo:jo + jw], ps[:, :jw],
                                         Act.Sqrt, bias=eps_col[0:1, :], scale=1.0 / H)
                    nc.vector.reciprocal(rrow[0:1, jo:jo + jw],
                                         sroot[0:1, jo:jo + jw])
                rrow_r = bc.tile([1, LP], F32R, name="rrow_r")
                nc.scalar.copy(rrow_r[:], rrow[:])
                rbc = bc.tile([P, LP], F32, name="rbc")
                for jo, jw in JT:
                    psb = pb.tile([P, 512], F32, tag="bc", name="bc1")
                    nc.tensor.matmul(psb[:, :jw], ones_row[:],
                                     rrow_r[0:1, jo:jo + jw],
                                     start=True, stop=True)
                    nc.scalar.copy(rbc[:, jo:jo + jw], psb[:, :jw])
                for k in range(KH):
                    nc.vector.tensor_mul(XH[k][:], X1[k][:], rbc[:])
                # r as per-token columns [128, NT] via tiny transposes
                ptt = ptr.tile([P, NT], F32, tag="rt", name="rt")
                for tb in range(NT):
                    nc.tensor.transpose(ptt[:, tb:tb + 1],
                                        rrow[0:1, tb * P:(tb + 1) * P],
                                        ident[0:1, 0:1])
                nc.scalar.copy(r_cols[:], ptt[:])

            # ---------------- phase F: router gating ----------------
            wbcp = open_pool("wbc", bufs=1, side="right")
            WBC = [wbcp.tile([P, LP], BF16, name=f"wbc{e}") for e in range(E)]
            wrows = wbcp.tile([E, LP], F32R, name="wrows")
            # broadcast-source rows live at base partitions 0/32/64 (matmul rule)
            wrow_t = [wbcp.tile([65, LP], F32R, name=f"wrt{i}") for i in range(3)]
            wrow_e = [wrow_t[e // 3][32 * (e % 3):32 * (e % 3) + 1, :] for e in range(E)]
            with ExitStack() as ph:
                wp = ph.enter_context(tc.tile_pool(name="wgate", bufs=1))
                gp = ph.enter_context(tc.tile_pool(name="gating", bufs=4))
                pg = ph.enter_context(tc.tile_pool(name="psg", bufs=4, space="PSUM"))
                pt_ = ph.enter_context(tc.tile_pool(name="psgt", bufs=2, space="PSUM"))
                pwb = ph.enter_context(tc.tile_pool(name="pswb", bufs=2, space="PSUM"))
                wgt_sb = []
                for k in range(KH):
                    t = wp.tile([P, E], F32, name=f"wgt{k}")
                    nc.sync.dma_start(t[:], wgt[k * P:(k + 1) * P, :])
                    wgt_sb.append(t)
                for tb in range(NT):
                    pg_t = pg.tile([P, E], F32, tag="g", name="g")
                    for k in range(KH):
                        nc.tensor.matmul(pg_t[:], X1[k][:, tb * P:(tb + 1) * P], wgt_sb[k][:],
                                         start=(k == 0), stop=(k == KH - 1))
                    s_t = gp.tile([P, E], F32, tag="s", name="s")
                    nc.scalar.activation(s_t[:], pg_t[:], Act.Exp,
                                         scale=r_cols[:, tb:tb + 1])
                    m1 = gp.tile([P, 1], F32, tag="m1", name="m1")
                    nc.vector.reduce_max(m1[:], s_t[:], axis=AX.X)
                    ml = gp.tile([P, E], F32, tag="ml", name="ml")
                    nc.vector.tensor_scalar(ml[:], s_t[:], m1[:], None, op0=Alu.is_lt)
                    s2 = gp.tile([P, E], F32, tag="s2", name="s2")
                    nc.vector.tensor_mul(s2[:], s_t[:], ml[:])
                    m2 = gp.tile([P, 1], F32, tag="m2", name="m2")
                    nc.vector.reduce_max(m2[:], s2[:], axis=AX.X)
                    keep = gp.tile([P, E], F32, tag="keep", name="keep")
                    nc.vector.tensor_scalar(keep[:], s_t[:], m2[:], None, op0=Alu.is_ge)
                    ssum = gp.tile([P, 1], F32, tag="ssum", name="ssum")
                    nc.vector.tensor_add(ssum[:], m1[:], m2[:])
                    srec = gp.tile([P, 1], F32, tag="srec", name="srec")
                    nc.vector.reciprocal(srec[:], ssum[:])
                    wt = gp.tile([P, E], F32, tag="wt", name="wt")
                    nc.vector.scalar_tensor_tensor(wt[:], s_t[:], srec[:], keep[:],
                                                   op0=Alu.mult, op1=Alu.mult)
                    pt_t = pt_.tile([E, P], F32, tag="wtT", name="wtT")
                    nc.tensor.transpose(pt_t[:], wt[:], ident[:])
                    nc.scalar.copy(wrows[:, tb * P:(tb + 1) * P], pt_t[:])
                for e in range(E):
                    nc.sync.dma_start(wrow_e[e][:], wrows[e:e + 1, :])
                for e in range(E):
                    for jo, jw in JT:
                        pw_t = pwb.tile([P, 512], F32, tag="wbc", name="wbcp")
                        base = 32 * (e % 3)
                        nc.tensor.matmul(pw_t[:, :jw], ones_bc[base:base + 1, :],
                                         wrow_e[e][0:1, jo:jo + jw],
                                         start=True, stop=True)
                        nc.scalar.copy(WBC[e][:, jo:jo + jw], pw_t[:, :jw])
            es["x1"].close()

            # ---------------- phase G: routed expert gate/up ----------------
            ap_ = open_pool("acts", bufs=1)
            A = [ap_.tile([P, LP], BF16, name=f"a{i}") for i in range(2 * E)]
            ASH = [ap_.tile([P, LP], BF16, name=f"ash{i}") for i in range(ISZ // P)]
            with ExitStack() as ph:
                tmp = ph.enter_context(tc.tile_pool(name="tmpgu", bufs=2))
                wst = ph.enter_context(tc.tile_pool(name="wgus", bufs=24))
                pp = ph.enter_context(tc.tile_pool(name="psgu", bufs=8, space="PSUM"))
                for fb in range(2 * E):
                    e = fb // 2
                    wgf = []
                    for k in range(KH):
                        t = wst.tile([P, P], BF16, tag="wgs", name="wgs")
                        nc.sync.dma_start(t[:], wgu_g[k * P:(k + 1) * P,
                                                      fb * P:(fb + 1) * P])
                        wgf.append(t)
                    wuf = []
                    for k in range(KH):
                        t = wst.tile([P, P], BF16, tag="wus", name="wus")
                        nc.sync.dma_start(t[:], wgu_g[k * P:(k + 1) * P,
                                                      E * I + fb * P:E * I + (fb + 1) * P])
                        wuf.append(t)
                    pg_ = [pp.tile([P, 512], F32, tag="gu", name="gu") for _ in JT]
                    for k in range(KH):
                        for j, (jo, jw) in enumerate(JT):
                            nc.tensor.matmul(pg_[j][:, :jw], wgf[k][:],
                                             XH[k][:, jo:jo + jw],
                                             start=(k == 0), stop=(k == KH - 1))
                    sgm = tmp.tile([P, LP], BF16, tag="sgm", name="sgm")
                    for j, (jo, jw) in enumerate(JT):
                        nc.scalar.activation(sgm[:, jo:jo + jw], pg_[j][:, :jw],
                                             Act.Sigmoid)
                    sg = tmp.tile([P, LP], BF16, tag="sg", name="sg")
                    for j, (jo, jw) in enumerate(JT):
                        nc.vector.tensor_mul(sg[:, jo:jo + jw], pg_[j][:, :jw],
                                             sgm[:, jo:jo + jw])
                    pu_ = [pp.tile([P, 512], F32, tag="gu", name="gu") for _ in JT]
                    for k in range(KH):
                        for j, (jo, jw) in enumerate(JT):
                            nc.tensor.matmul(pu_[j][:, :jw], wuf[k][:],
                                             XH[k][:, jo:jo + jw],
                                             start=(k == 0), stop=(k == KH - 1))
                    ta = tmp.tile([P, LP], BF16, tag="ta", name="ta")
                    for j, (jo, jw) in enumerate(JT):
                        nc.vector.tensor_mul(ta[:, jo:jo + jw], pu_[j][:, :jw],
                                             sg[:, jo:jo + jw])
                    nc.vector.tensor_mul(A[fb][:], ta[:], WBC[e][:])
            es["wbc"].close()

            # down-proj weights prefetch (DMA overlaps shared expert phase)
            wdp = open_pool("wd", bufs=1)
            wd_sb = []
            for k in range(NKD):
                t = wdp.tile([P, H], BF16, name=f"wd{k}")
                nc.sync.dma_start(t[:], wd_g[k * P:(k + 1) * P, :])
                wd_sb.append(t)

            # ---------------- phase H: shared expert gate/up ----------------
            with ExitStack() as ph:
                tmp = ph.enter_context(tc.tile_pool(name="tmpsgu", bufs=2))
                pp = ph.enter_context(tc.tile_pool(name="pssgu", bufs=8, space="PSUM"))
                for fb in range(ISZ // P):
                    pg_ = [pp.tile([P, 512], F32, tag="sgu", name="sgu") for _ in JT]
                    for k in range(KH):
                        for j, (jo, jw) in enumerate(JT):
                            nc.tensor.matmul(pg_[j][:, :jw],
                                             wsg_sb[k][:, fb * P:(fb + 1) * P],
                                             XH[k][:, jo:jo + jw],
                                             start=(k == 0), stop=(k == KH - 1))
                    sgm = tmp.tile([P, LP], BF16, tag="ssgm", name="ssgm")
                    for j, (jo, jw) in enumerate(JT):
                        nc.scalar.activation(sgm[:, jo:jo + jw], pg_[j][:, :jw],
                                             Act.Sigmoid)
                    sg = tmp.tile([P, LP], BF16, tag="ssg", name="ssg")
                    for j, (jo, jw) in enumerate(JT):
                        nc.vector.tensor_mul(sg[:, jo:jo + jw], pg_[j][:, :jw],
                                             sgm[:, jo:jo + jw])
                    pu_ = [pp.tile([P, 512], F32, tag="sgu", name="sgu") for _ in JT]
                    for k in range(KH):
                        for j, (jo, jw) in enumerate(JT):
                            nc.tensor.matmul(pu_[j][:, :jw],
                                             wsu_sb[k][:, fb * P:(fb + 1) * P],
                                             XH[k][:, jo:jo + jw],
                                             start=(k == 0), stop=(k == KH - 1))
                    for j, (jo, jw) in enumerate(JT):
                        nc.vector.tensor_mul(ASH[fb][:, jo:jo + jw], pu_[j][:, :jw],
                                             sg[:, jo:jo + jw])
            es["xhat"].close()
            es["wexp"].close()

            # ------------- phase I: down proj (routed + shared fused) -------------
            yp = open_pool("y", bufs=1, side="right")
            Y = [yp.tile([P, LP], F32, name=f"y{i}") for i in range(KH)]
            YB = [yp.tile([P, LP], BF16, name=f"yb{i}") for i in range(KH)]
            AALL = A + ASH
            with ExitStack() as ph:
                pp = ph.enter_context(tc.tile_pool(name="psd", bufs=6, space="PSUM"))
                for hb in range(KH):
                    pts = [pp.tile([P, 512], F32, tag="y", name="yps") for _ in JT]
                    for k in range(NKD):
                        for j, (jo, jw) in enumerate(JT):
                            nc.tensor.matmul(pts[j][:, :jw],
                                             wd_sb[k][:, hb * P:(hb + 1) * P],
                                             AALL[k][:, jo:jo + jw],
                                             start=(k == 0), stop=(k == NKD - 1))
                    for j, (jo, jw) in enumerate(JT):
                        nc.scalar.copy(Y[hb][:, jo:jo + jw], pts[j][:, :jw])
                        nc.vector.tensor_copy(YB[hb][:, jo:jo + jw], pts[j][:, :jw])
            es["wd"].close()
            es["acts"].close()

            # ---------------- phase J: output gate + final mask ----------------
            with ExitStack() as ph:
                wp = ph.enter_context(tc.tile_pool(name="wog", bufs=1))
                fr = ph.enter_context(tc.tile_pool(name="final", bufs=1))
                op_ = ph.enter_context(tc.tile_pool(name="outp", bufs=3))
                pg = ph.enter_context(tc.tile_pool(name="psog", bufs=2, space="PSUM"))
                pbf = ph.enter_context(tc.tile_pool(name="psfin", bufs=1, space="PSUM"))
                ogc_sb = wp.tile([P, KH], BF16, name="ogc")
                nc.sync.dma_start(ogc_sb[:], ogm[:, :])
                ogb_sb = wp.tile([1, 1], F32, name="ogb")
                nc.sync.dma_start(ogb_sb[:], ogb[:, :])
                sigrow = fr.tile([1, LP], F32, name="sigrow")
                for jo, jw in JT:
                    pg_t = pg.tile([1, 512], F32, tag="og", name="og")
                    for k in range(KH):
                        nc.tensor.matmul(pg_t[:, :jw], ogc_sb[:, k:k + 1],
                                         YB[k][:, jo:jo + jw],
                                         start=(k == 0), stop=(k == KH - 1))
                    nc.scalar.activation(sigrow[0:1, jo:jo + jw], pg_t[:, :jw],
                                         Act.Sigmoid, bias=ogb_sb[0:1, :])
                svrow = fr.tile([1, LP], F32R, name="svrow")
                nc.vector.tensor_mul(svrow[:], sigrow[:], valid[:])
                svb = fr.tile([P, LP], F32, name="svb")
                for jo, jw in JT:
                    pb_t = pbf.tile([P, 512], F32, tag="fin", name="fin")
                    nc.tensor.matmul(pb_t[:, :jw], ones_row[:],
                                     svrow[0:1, jo:jo + jw],
                                     start=True, stop=True)
                    nc.scalar.copy(svb[:, jo:jo + jw], pb_t[:, :jw])
                for hb in range(KH):
                    ot = op_.tile([P, LP], F16, tag="ot", name="ot")
                    nc.vector.tensor_mul(ot[:], Y[hb][:], svb[:])
                    nc.sync.dma_start(outm[hb * P:(hb + 1) * P, :], ot[:])
            es["y"].close()

    nc.compile()
    return nc


_CACHE = {}


def _get_program(NT):
    if NT not in _CACHE:
        _CACHE[NT] = build(NT)
    return _CACHE[NT]


def _prep_inputs(inputs, NT):
    f32 = np.float32
    f16 = np.float16
    bf = ml_dtypes.bfloat16
    LP = NT * P
    g = lambda k: np.asarray(inputs[k]).astype(f32, copy=False)

    hs = g("hidden_states")
    tcs = np.asarray(inputs["true_counts"]).astype(np.int64).reshape(B)
    cnw, gnw, snw = g("context_norm_w"), g("gate_norm_w"), g("shared_norm_w")
    ipw, ipb = g("in_proj_w"), g("in_proj_b")
    opw, opb = g("out_proj_w"), g("out_proj_b")
    gw = g("gate_w")
    enw = g("expert_norm_w")
    egw, euw, edw = g("expert_gate_w"), g("expert_up_w"), g("expert_down_w")
    sgw, suw, sdw = g("shared_gate_w"), g("shared_up_w"), g("shared_down_w")
    ogw, ogb_ = g("out_gate_w"), g("out_gate_b")

    wattn = np.empty((H, 4 * H), f16)
    wattn[:, :3 * H] = (ipw * cnw[None, :]).T
    wattn[:, 3 * H:] = opw.T
    wgu = np.empty((H, 2 * E * I), bf)
    wgu[:, :E * I] = (egw * enw[:, None, :]).reshape(E * I, H).T
    wgu[:, E * I:] = (euw * enw[:, None, :]).reshape(E * I, H).T
    wd = np.empty((WD_ROWS, H), bf)
    wd[:E * I] = edw.transpose(0, 2, 1).reshape(E * I, H)
    wd[E * I:] = sdw.T
    wsgu = np.empty((H, 2 * ISZ), bf)
    wsgu[:, :ISZ] = (sgw * snw[None, :]).T
    wsgu[:, ISZ:] = (suw * snw[None, :]).T

    wdr = WD_ROWS // NCORES
    shared_small = {
        "wgateT": np.ascontiguousarray((gw * gnw[None, :]).T),
        "ogc": np.ascontiguousarray(ogw.reshape(KH, P).T.astype(bf)),
        "ogb": ogb_.reshape(1, 1),
        "bqk": np.ascontiguousarray(ipb[:2 * H].reshape(16, P).T),
        "bv_row": np.ascontiguousarray(ipb[2 * H:].reshape(1, H).astype(f16)),
        "bop": np.ascontiguousarray(opb.reshape(KH, P).T),
    }
    in_maps = []
    for b in range(B):
        m = dict(shared_small)
        m["x_t"] = hs[b, :LP].T.astype(f16)
        m["tc_col"] = np.full((P, 1), float(tcs[b]), f32)
        m["wattn_sh"] = wattn[b * P:(b + 1) * P]
        m["wgu_sh"] = wgu[b * P:(b + 1) * P]
        m["wd_sh"] = wd[b * wdr:(b + 1) * wdr]
        m["wsgu_sh"] = wsgu[b * P:(b + 1) * P]
        in_maps.append(m)
    return in_maps


LAST_RESULT = None


def _run(inputs, **kw):
    global LAST_RESULT
    tcs = np.asarray(inputs["true_counts"]).astype(np.int64).reshape(B)
    NT = min(KH, max(1, int(-(-int(tcs.max()) // P))))
    LP = NT * P
    nc = _get_program(NT)
    in_maps = _prep_inputs(inputs, NT)
    res = run_bass_kernel_spmd(nc, in_maps, core_ids=list(range(B)), **kw)
    LAST_RESULT = res
    out = np.zeros((B, L, H), np.float32)
    for b in range(B):
        out[b, :LP] = res.results[b]["out"].T
    return out


def kernel(**inputs):
    return _run(inputs)
